# revision 3
# baseline (speedup 1.0000x reference)
"""Trainium2 Bass kernel for nn_MHA_34050500723480 — v2.

MHA forward: out = softmax((x@Wq)(x@Wk)^T / 128 + mask*-1e9) @ (x@Wv) @ W_out

Sharding: 8 cores = 2 batches x 4 head-groups (4 heads of dim 128 each).
Each core computes its batch's attention for its 4 heads plus the
row-parallel slice of out_proj; host sums the 4 partial out_proj results
per batch and adds the (v-bias @ W_out + b_out) constant.

v2 key ideas vs baseline:
- Masked-KEY compaction: the mask only excludes keys, so k/v projection,
  scores, PV and den only run over the ~1024 unmasked tokens (host
  gathers them into xkv, padded to NKV mult of 128). Padding keys are
  killed with a -30 bias inside the exp activation (exp ~ 1e-13), so no
  v-zeroing pass and no zt mask tensor is needed.
- All matmul operands bf16 (same PE rate as f32r, half DMA + SBUF).
- v stays in SBUF (no DRAM spill/reload).
- Out partials written bf16 (half the output DMA).

v3 on top of v2:
- Softmax denominator moved off the PE: exp-tile sums on DVE+gpsimd
  (4-op tree) + one ones-column matmul per (head, q-chunk) instead of
  one den matmul per key block (saves ~65K PE cycles/core).
- exp computed on [128, 2, 512] pair tiles (halves ACT instruction
  overhead); the last 2-3 key blocks run as singles with the pad bias.
- DMA order fixed: all x chunks precede xkv on the sync queue (A1 was
  DMA-starved); wo/out DMA moved to the sync queue so the scalar
  engine's ACT stream is undisturbed.
- Phase C psum->sbuf copies split between ACT and DVE.

v4 on top of v3:
- q/k projections in fp8e4m3 with DoubleRow perf mode (0.5 PE
  cycles/row). Softmax logits are tiny (std ~0.03) so the ~3%
  elementwise fp8 error on q/k moves probabilities by only ~0.2%.
  x and Wq/Wk are pre-scaled on the host (SX*SW = 2^16) and descaled
  in the bias-add tensor_scalar.
- v projection (accuracy-critical) keeps the bf16 xkv copy.

v5 on top of v4:
- Warmup dummies for gpsimd (partition_broadcast), the custom-DVE
  reciprocal and the ACT Exp table at kernel start: the first dispatch
  of each costs ~1-7us and was stalling unit 0's softmax finalize.
- First wq/xc DMAs split finer so the PE starts ~5us earlier.
"""

import os
import sys

import numpy as np

for _p in ("/opt/trn_rl_repo",):
    if os.path.isdir(_p) and _p not in sys.path:
        sys.path.insert(0, _p)

# Problem shapes (hardcoded per contract).
B = 2
S = 2048
E = 2048
D = 128          # head dim
HPC = 4          # heads per core
W = HPC * D      # 512: per-core width of q/k/v
ET = E // 128    # 16 contraction tiles for proj
SC = S // 512    # 4 x-chunks for q proj
QC = S // 512    # 4 q-chunks in attention / out proj
EB = E // 128    # 16 output e-blocks
CT = W // 128    # 4 contraction tiles for out proj
PAD_BIAS = -30.0
SX = 16.0       # fp8 scale for x
SW = 4096.0     # fp8 scale for Wq/Wk
DESCALE = 1.0 / (SX * SW)

_CACHE = {}


def _build_nc(NKV):
    """Build (once per NKV) the single-core Bass/Tile program for all 8 cores."""
    from contextlib import ExitStack

    import concourse.bass as bass  # noqa: F401  (import side effects)
    import concourse.mybir as mybir
    import concourse.tile as tile
    from concourse import bacc

    dt = mybir.dt
    f32 = dt.float32
    bf16 = dt.bfloat16
    Exp = mybir.ActivationFunctionType.Exp

    TBK = NKV // 128       # key blocks
    KCH = 512              # k-proj moving chunk
    KC = (NKV + KCH - 1) // KCH

    nc = bacc.Bacc("TRN2", target_bir_lowering=False, debug=False, num_devices=8)

    f8 = dt.float8e4
    DR = mybir.MatmulPerfMode.DoubleRow
    xc_d = nc.dram_tensor("xc", (SC, 128, ET, 512), f8, kind="ExternalInput").ap()
    xkv_d = nc.dram_tensor("xkv", (128, ET, NKV), bf16, kind="ExternalInput").ap()
    xk8_d = nc.dram_tensor("xk8", (128, ET, NKV), f8, kind="ExternalInput").ap()
    wq_d = nc.dram_tensor("wq", (HPC, 128, ET, 128), f8, kind="ExternalInput").ap()
    wk_d = nc.dram_tensor("wk", (HPC, 128, ET, 128), f8, kind="ExternalInput").ap()
    wv_d = nc.dram_tensor("wv", (128, ET, W), bf16, kind="ExternalInput").ap()
    wo_d = nc.dram_tensor("wo", (EB, 128, CT, 128), bf16, kind="ExternalInput").ap()
    bq_d = nc.dram_tensor("bq", (128, HPC), f32, kind="ExternalInput").ap()
    bk_d = nc.dram_tensor("bk", (128, HPC), f32, kind="ExternalInput").ap()
    padb_d = nc.dram_tensor("padb", (128, TBK), f32, kind="ExternalInput").ap()
    ones_d = nc.dram_tensor("ones", (128, 1), bf16, kind="ExternalInput").ap()
    out_d = nc.dram_tensor("out", (EB, 128, S), bf16, kind="ExternalOutput").ap()

    with tile.TileContext(nc) as tc, ExitStack() as top:
        const = top.enter_context(tc.tile_pool(name="const", bufs=1))
        persist = top.enter_context(tc.tile_pool(name="persist", bufs=1))

        bq_t = const.tile([128, HPC], f32)
        nc.scalar.dma_start(bq_t[:], bq_d[:])
        bk_t = const.tile([128, HPC], f32)
        nc.scalar.dma_start(bk_t[:], bk_d[:])
        padb_t = const.tile([128, TBK], f32)
        nc.scalar.dma_start(padb_t[:], padb_d[:])
        ones_t = const.tile([128, 1], bf16)
        nc.scalar.dma_start(ones_t[:], ones_d[:])

        # Warmups: first dispatch of the gpsimd broadcast ucode, the
        # custom-DVE reciprocal and the ACT Exp table each cost 1-7us;
        # pay them here instead of in phase B's first softmax.
        wrm = const.tile([1, 8], f32)
        nc.vector.memset(wrm[:], 1.0)
        wrm2 = const.tile([1, 8], f32)
        nc.vector.reciprocal_approx_fast(wrm2[:], wrm[:])
        wrmb = const.tile([128, 8], f32)
        nc.gpsimd.partition_broadcast(wrmb[:], wrm2[:])
        wrme = const.tile([1, 8], bf16)
        nc.scalar.activation(wrme[:], wrm[:], Exp, scale=1.0)

        qT = persist.tile([128, HPC, S], bf16)     # q^T per head: [d, s]
        kT = persist.tile([128, HPC, NKV], bf16)   # k^T per head: [d, kv]
        vsb = persist.tile([128, TBK, W], bf16)    # v: [kv-block, d(4 heads)]
        ctx_sb = persist.tile([128, HPC, S], bf16)  # context^T per head [d, q]

        # ---------------- Phase A: qkv projection ----------------
        with ExitStack() as pa:
            wqk_pool = pa.enter_context(tc.tile_pool(name="wqk", bufs=1))
            wv_pool = pa.enter_context(tc.tile_pool(name="wv", bufs=1))
            xkv_pool = pa.enter_context(tc.tile_pool(name="xkv", bufs=1))
            xpool = pa.enter_context(tc.tile_pool(name="xc", bufs=2))
            qk_ps = pa.enter_context(tc.tile_pool(name="qkps", bufs=4, space="PSUM"))
            v_ps = pa.enter_context(tc.tile_pool(name="vps", bufs=2, space="PSUM"))

            # Weights on the scalar queue; wq first (h=0 split so the very
            # first et-pair lands in ~1us) so MMs can start early.
            wq_res = []
            for h in range(HPC):
                t = wqk_pool.tile([128, ET, 128], f8, tag=f"wq{h}",
                                  name=f"wq_res{h}")
                if h == 0:
                    nc.scalar.dma_start(t[:, 0:2], wq_d[h][:, 0:2])
                    nc.scalar.dma_start(t[:, 2:], wq_d[h][:, 2:])
                else:
                    nc.scalar.dma_start(t[:], wq_d[h])
                wq_res.append(t)
            wk_res = []
            for h in range(HPC):
                t = wqk_pool.tile([128, ET, 128], f8, tag=f"wk{h}",
                                  name=f"wk_res{h}")
                nc.scalar.dma_start(t[:], wk_d[h])
                wk_res.append(t)
            wv_t = wv_pool.tile([128, ET, W], bf16)
            nc.scalar.dma_start(wv_t[:], wv_d[:])

            # x chunks on the sync queue (fine-grained first chunk for fast
            # start); xkv on the scalar queue behind the weights so it never
            # delays the x chunks A1 is consuming.
            xtiles = {}

            def load_chunk(sc):
                xt = xpool.tile([128, ET, 512], f8, tag="xc", name=f"xt_{sc}")
                if sc == 0:
                    for et in range(ET):
                        nc.sync.dma_start(xt[:, et], xc_d[sc, :, et])
                else:
                    nc.sync.dma_start(xt[:], xc_d[sc])
                xtiles[sc] = xt

            load_chunk(0)
            load_chunk(1)
            # Gate the big xk8/xkv loads behind the x chunks A1 is eating:
            # ungated they oversubscribe HBM (~430GB/s demand vs 358) and
            # starve A1. The dummy ACT read of chunk 1 delays the scalar
            # queue until the early x stream has landed.
            gate = const.tile([1, 8], f8, tag="gate")
            nc.scalar.copy(gate[:], xtiles[1][0:1, 0, 0:8])
            xk8_t = xkv_pool.tile([128, ET, NKV], f8, tag="xk8")
            for et in range(0, ET, 4):
                nc.scalar.dma_start(xk8_t[:, et:et + 4], xk8_d[:, et:et + 4])
            xkv_t = xkv_pool.tile([128, ET, NKV], bf16)
            for et in range(0, ET, 4):
                nc.scalar.dma_start(xkv_t[:, et:et + 4], xkv_d[:, et:et + 4])

            ts_mult = mybir.AluOpType.mult
            ts_add = mybir.AluOpType.add

            # A1: q projection over all of x (fp8 DoubleRow over et pairs,
            # N=1024 moving so the 256-col LDW is hidden)
            for sc in range(SC):
                if sc + 2 < SC:
                    load_chunk(sc + 2)
                xt = xtiles.pop(sc)
                s0 = sc * 512
                for h in range(HPC):
                    ps = qk_ps.tile([128, 512], f32, tag="qk")
                    for t in range(ET // 2):
                        nc.tensor.matmul(
                            ps[:],
                            wq_res[h][:, 2 * t:2 * t + 2],
                            xt[:, 2 * t:2 * t + 2],
                            start=(t == 0),
                            stop=(t == ET // 2 - 1),
                            perf_mode=DR,
                        )
                    nc.vector.tensor_scalar(
                        qT[:, h, s0:s0 + 512], ps[:], DESCALE,
                        bq_t[:, h:h + 1], ts_mult, ts_add,
                    )

            # A2: k projection over compacted keys (fp8 DoubleRow)
            for kc in range(KC):
                k0 = kc * KCH
                kw = min(KCH, NKV - k0)
                for h in range(HPC):
                    ps = qk_ps.tile([128, 512], f32, tag="qk")
                    for t in range(ET // 2):
                        nc.tensor.matmul(
                            ps[:, :kw],
                            wk_res[h][:, 2 * t:2 * t + 2],
                            xk8_t[:, 2 * t:2 * t + 2, k0:k0 + kw],
                            start=(t == 0),
                            stop=(t == ET // 2 - 1),
                            perf_mode=DR,
                        )
                    nc.vector.tensor_scalar(
                        kT[:, h, k0:k0 + kw], ps[:, :kw], DESCALE,
                        bk_t[:, h:h + 1], ts_mult, ts_add,
                    )

            # A3: v projection (no bias: host folds b_v@W_out into the output)
            for kb in range(TBK):
                ps = v_ps.tile([128, W], f32, tag="v")
                for et in range(ET):
                    nc.tensor.matmul(
                        ps[:],
                        xkv_t[:, et, kb * 128:(kb + 1) * 128],
                        wv_t[:, et],
                        start=(et == 0),
                        stop=(et == ET - 1),
                    )
                nc.scalar.copy(vsb[:, kb], ps[:])

        # wout stream pool opened before phase B so its DMAs prefetch during B
        wo_pool = top.enter_context(tc.tile_pool(name="wo", bufs=6))

        # ---------------- Phase B: attention per head ----------------
        # Key blocks are processed as 3 pair-groups (blocks 0-5, one exp
        # ACT each, zero bias) + singles for blocks 6..TBK-1 (pad bias AP).
        # The denominator never touches the PE per-block: exp tiles are
        # summed on DVE+gpsimd (4-5 op tree) and one ones-column matmul per
        # unit reduces across partitions.
        assert TBK in (8, 9), TBK
        NSING = TBK - 6
        add = mybir.AluOpType.add

        with ExitStack() as pb:
            exp_pool = pb.enter_context(tc.tile_pool(name="exp", bufs=3))
            dt_pool = pb.enter_context(tc.tile_pool(name="dtree", bufs=2))
            rep_pool = pb.enter_context(tc.tile_pool(name="rep", bufs=2))
            rc_pool = pb.enter_context(tc.tile_pool(name="recip", bufs=2))
            sc_ps = pb.enter_context(tc.tile_pool(name="scps", bufs=2, space="PSUM"))
            ctx_ps = pb.enter_context(tc.tile_pool(name="ctxps", bufs=2, space="PSUM"))
            den_ps = pb.enter_context(tc.tile_pool(name="denps", bufs=2, space="PSUM"))

            reduce_prev = None
            finalize_prev = None
            for h in range(HPC):
                for qc in range(QC):
                    q0 = qc * 512
                    ctxp = ctx_ps.tile([128, 512], f32, tag="ctx")

                    def scores_pair(i0, n, h=h, q0=q0):
                        sp = sc_ps.tile([128, 2, 512], f32, tag="sc")
                        for j in range(n):
                            kb = i0 + j
                            nc.tensor.matmul(
                                sp[:, j], kT[:, h, kb * 128:(kb + 1) * 128],
                                qT[:, h, q0:q0 + 512], start=True, stop=True,
                            )
                        return sp

                    def emit_pv(ex, i0, n, ctxp=ctxp, h=h):
                        for j in range(n):
                            kb = i0 + j
                            nc.tensor.matmul(
                                ctxp[:],
                                vsb[:, kb, h * 128:(h + 1) * 128],
                                ex[:, j],
                                start=(kb == 0),
                                stop=(kb == TBK - 1),
                            )

                    # --- group 0 (blocks 0,1) ---
                    sp0 = scores_pair(0, 2)
                    # Delayed tail of the previous unit goes here so the PE
                    # never waits on the DVE/gpsimd den tree.
                    if reduce_prev is not None:
                        reduce_prev()
                        finalize_prev()
                        reduce_prev = finalize_prev = None
                    ex0 = exp_pool.tile([128, 2, 512], bf16, tag="exp0")
                    nc.scalar.activation(ex0[:], sp0[:], Exp, scale=1.0 / D)
                    # --- group 1 (blocks 2,3) ---
                    sp1 = scores_pair(2, 2)
                    ex1 = exp_pool.tile([128, 2, 512], bf16, tag="exp1")
                    nc.scalar.activation(ex1[:], sp1[:], Exp, scale=1.0 / D)
                    d12 = dt_pool.tile([128, 2, 512], bf16, tag="d12")
                    nc.vector.tensor_tensor(d12[:], ex0[:], ex1[:], add)
                    emit_pv(ex0, 0, 2)
                    # --- group 2 (blocks 4,5) ---
                    sp2 = scores_pair(4, 2)
                    ex2 = exp_pool.tile([128, 2, 512], bf16, tag="exp2")
                    nc.scalar.activation(ex2[:], sp2[:], Exp, scale=1.0 / D)
                    emit_pv(ex1, 2, 2)
                    # --- singles (blocks 6..TBK-1, pad bias) ---
                    exl = exp_pool.tile([128, NSING, 512], bf16, tag="exl")
                    sps = sc_ps.tile([128, 2, 512], f32, tag="sc")
                    for j in range(2):
                        kb = 6 + j
                        nc.tensor.matmul(
                            sps[:, j], kT[:, h, kb * 128:(kb + 1) * 128],
                            qT[:, h, q0:q0 + 512], start=True, stop=True,
                        )
                        nc.scalar.activation(
                            exl[:, j], sps[:, j], Exp,
                            bias=padb_t[:, kb:kb + 1], scale=1.0 / D,
                        )
                    if NSING == 3:
                        sps2 = sc_ps.tile([128, 2, 512], f32, tag="sc")
                        nc.tensor.matmul(
                            sps2[:, 0], kT[:, h, 8 * 128:9 * 128],
                            qT[:, h, q0:q0 + 512], start=True, stop=True,
                        )
                        nc.scalar.activation(
                            exl[:, 2], sps2[:, 0], Exp,
                            bias=padb_t[:, 8:9], scale=1.0 / D,
                        )
                    emit_pv(ex2, 4, 2)
                    # den tree: all on DVE in bf16 (2x rate, no casts); gpsimd
                    # is too slow for these adds (measured 2.2us each).
                    d34 = dt_pool.tile([128, 2, 512], bf16, tag="d34")
                    nc.vector.tensor_tensor(d34[:], ex2[:], exl[:, 0:2], add)
                    ee = dt_pool.tile([128, 2, 512], bf16, tag="ee")
                    nc.vector.tensor_tensor(ee[:], d12[:], d34[:], add)
                    den_acc = dt_pool.tile([128, 512], bf16, tag="dacc")
                    if NSING == 2:
                        nc.vector.tensor_tensor(
                            den_acc[:], ee[:, 0], ee[:, 1], add)
                    else:
                        ff = dt_pool.tile([128, 512], bf16, tag="ff")
                        nc.vector.tensor_tensor(ff[:], ee[:, 0], ee[:, 1], add)
                        nc.vector.tensor_tensor(
                            den_acc[:], ff[:], exl[:, 2], add)
                    emit_pv(exl, 6, NSING)

                    def reduce(den_acc=den_acc):
                        denp = den_ps.tile([1, 512], f32, tag="den")
                        nc.tensor.matmul(
                            denp[:], ones_t[:], den_acc[:],
                            start=True, stop=True,
                        )
                        reduce.denp = denp

                    def finalize(ctxp=ctxp, h=h, q0=q0, reduce=reduce):
                        denp = reduce.denp
                        rc = rc_pool.tile([1, 512], f32, tag="rc")
                        nc.vector.reciprocal_approx_fast(rc[:], denp[:])
                        rs = rep_pool.tile([128, 512], f32, tag="rep")
                        nc.gpsimd.partition_broadcast(rs[:], rc[:])
                        nc.vector.tensor_tensor(
                            ctx_sb[:, h, q0:q0 + 512], ctxp[:], rs[:],
                            mybir.AluOpType.mult,
                        )

                    reduce_prev = reduce
                    finalize_prev = finalize
            reduce_prev()
            finalize_prev()

        # ---------------- Phase C: out projection (row-parallel partial) ----------------
        with ExitStack() as pc:
            ob_pool = pc.enter_context(tc.tile_pool(name="ob", bufs=3))
            o_ps = pc.enter_context(tc.tile_pool(name="ops", bufs=4, space="PSUM"))

            wo_tiles = {}

            def load_wo(eb):
                wo_t = wo_pool.tile([128, CT, 128], bf16, tag="wo",
                                    name=f"wo_{eb}")
                nc.sync.dma_start(wo_t[:], wo_d[eb])
                wo_tiles[eb] = wo_t

            load_wo(0)
            for eb in range(EB):
                if eb + 1 < EB:
                    load_wo(eb + 1)
                wo_t = wo_tiles.pop(eb)
                ob = ob_pool.tile([128, QC, 512], bf16, tag="ob")
                for qc in range(QC):
                    q0 = qc * 512
                    op = o_ps.tile([128, 512], f32, tag="o")
                    for ct in range(CT):
                        nc.tensor.matmul(
                            op[:],
                            wo_t[:, ct],
                            ctx_sb[:, ct, q0:q0 + 512],
                            start=(ct == 0),
                            stop=(ct == CT - 1),
                        )
                    if qc % 2 == 0:
                        nc.scalar.copy(ob[:, qc], op[:])
                    else:
                        nc.vector.tensor_copy(ob[:, qc], op[:])
                nc.sync.dma_start(out_d[eb], ob[:])

    nc.compile()
    return nc


def get_nc(NKV):
    key = ("nc", NKV)
    if key not in _CACHE:
        _CACHE[key] = _build_nc(NKV)
    return _CACHE[key]


def _bf16(a):
    import ml_dtypes
    return np.ascontiguousarray(a, dtype=np.float32).astype(ml_dtypes.bfloat16)


def _f8(a, scale):
    import ml_dtypes
    a = np.ascontiguousarray(a, dtype=np.float32) * np.float32(scale)
    np.clip(a, -240.0, 240.0, out=a)
    return a.astype(ml_dtypes.float8_e4m3)


def shard_inputs(c, x, mask, W_qkv, b_qkv, W_out, NKV):
    """Per-core input map, laid out so every device DMA is linear."""
    b, g = divmod(c, 4)
    TBK = NKV // 128
    keep = np.flatnonzero(mask[b] == 0.0)
    n = len(keep)

    xT = np.ascontiguousarray(x[b].T)  # [E, S]
    xc = _f8(xT.reshape(ET, 128, SC, 512).transpose(2, 1, 0, 3), SX)
    xkvT = np.zeros((E, NKV), np.float32)
    xkvT[:, :n] = xT[:, keep]
    xkvr = xkvT.reshape(ET, 128, NKV).transpose(1, 0, 2)
    xkv = _bf16(xkvr)
    xk8 = _f8(xkvr, SX)

    qs = W_qkv[:, g * W:(g + 1) * W]
    ks = W_qkv[:, E + g * W:E + (g + 1) * W]
    vs = W_qkv[:, 2 * E + g * W:2 * E + (g + 1) * W]
    wq = _f8(qs.reshape(ET, 128, HPC, 128).transpose(2, 1, 0, 3), SW)
    wk = _f8(ks.reshape(ET, 128, HPC, 128).transpose(2, 1, 0, 3), SW)
    wv = _bf16(vs.reshape(ET, 128, W).transpose(1, 0, 2))
    wo = _bf16(
        W_out[g * W:(g + 1) * W, :]
        .reshape(CT, 128, EB, 128).transpose(2, 1, 0, 3)
    )
    bq = np.ascontiguousarray(b_qkv[g * W:(g + 1) * W].reshape(HPC, 128).T)
    bk = np.ascontiguousarray(
        b_qkv[E + g * W:E + (g + 1) * W].reshape(HPC, 128).T)
    padb = np.zeros((TBK, 128), np.float32)
    flat = padb.reshape(-1)
    flat[n:] = PAD_BIAS
    padb = np.ascontiguousarray(padb.T)
    ones = _bf16(np.ones((128, 1), np.float32))
    return dict(xc=xc, xkv=xkv, xk8=xk8, wq=wq, wk=wk, wv=wv, wo=wo,
                bq=bq, bk=bk, padb=padb, ones=ones)


def run(inputs, trace=False, trace_kwargs=None):
    """Run on 8 cores; returns (full output [B,S,E] f32, BassKernelResults)."""
    from concourse import bass_utils

    x = np.asarray(inputs["x"], dtype=np.float32)
    mask = np.asarray(inputs["mask"], dtype=np.float32)
    W_qkv = np.asarray(inputs["W_qkv"], dtype=np.float32)
    b_qkv = np.asarray(inputs["b_qkv"], dtype=np.float32)
    W_out = np.asarray(inputs["W_out"], dtype=np.float32)
    b_out = np.asarray(inputs["b_out"], dtype=np.float32)

    keeps = [int((mask[b] == 0.0).sum()) for b in range(B)]
    max_keep = max(keeps)
    NKV = max(128, -(-max_keep // 128) * 128)
    # pads must fall in the single-blocks (>= block 6) which carry the
    # pad-bias AP; blocks 0-5 are exp'ed with zero bias
    assert min(keeps) >= 6 * 128, keeps

    nc = get_nc(NKV)
    in_maps = [shard_inputs(c, x, mask, W_qkv, b_qkv, W_out, NKV)
               for c in range(8)]
    res = bass_utils.run_bass_kernel_spmd(
        nc, in_maps, core_ids=list(range(8)), trace=trace,
        **(trace_kwargs or {}),
    )

    out_full = np.zeros((B, S, E), np.float32)
    for c, r in enumerate(res.results):
        b, _g = divmod(c, 4)
        o = np.asarray(r["out"], dtype=np.float32)  # [EB, 128, S] = out^T partial
        out_full[b] += o.transpose(2, 0, 1).reshape(S, E)
    bv = b_qkv[2 * E:]
    out_full += (bv @ W_out + b_out)[None, None, :]
    return out_full, res


def kernel(**inputs) -> np.ndarray:
    return run(inputs, trace=False)[0]
